# revision 1
# baseline (speedup 1.0000x reference)
"""Trainium2 Bass kernel: nn_DifferentiableSelector (soft top-K w/ refractory damping).

Data-parallel over batch: 512 rows -> 64 rows/core on 8 NeuronCores.

Device layout ("two contiguous row-chunks"): each core's [64, 32768] block is
split into 2 contiguous address-range chunks of 32 rows. Chunk k, viewed as
[128, 4096], holds rows 32k..32k+31 with row 32k+j on partitions
[4j, 4j+4) — so every DMA is one fully-contiguous 4MB transfer (measured
6-30x faster on this target than partition-interleaved patterns), and chunk
k+1's input DMA overlaps chunk k's compute while chunk k's output DMA overlaps
chunk k+1's compute. Per chunk: sigmoid as 2048-wide out-of-place ACT tiles
with fused row-partial accumulation (accum_out), one PE matmul against a 0/1
block matrix to group-sum + broadcast the row budgets, reciprocal straight
from PSUM, then one full-width (even-length, 2x-mode) DVE tensor_scalar scale
pass; column 0 of each row is then overwritten via a masked per-partition
factor to implement y[:, 0] = 0.

Math: y0 = sigmoid(scores/temp); budget_r = clip(sum_i y0[r,i], 1e-6);
y = y0 * min(K/budget, 1); then R=4 damping iters
y *= min(2/(1+y+roll(y,-d)), 1); y[:,0] = 0.

Damping-identity property (load-bearing): if budget_r >= 2K = 128 for every
row, then min(K/budget,1) <= 0.5 (correctly-rounded fp32 div), so every
y <= 0.5, so s = fl(y[i]+y[i+d]) <= 1, fl(1+s) <= 2, fl(2/(1+s)) >= 1, and
min(2/(1+s), 1.0) == 1.0 *exactly*; y*1.0 is bitwise identity. Inductively the
whole damping loop is an exact fp32 no-op. For N(0,1)-like scores,
budget ~ T/2 = 16384 (margin ~128x over the threshold). The device exports the
raw per-row sums; the host checks sum >= 256 for every row and otherwise falls
back to a full numpy evaluation of the reference semantics (exact for
arbitrary inputs; never taken for the spec'd input distribution). The same
check makes clip(budget, 1e-6) and min(K/budget, 1) identities on the device
path, so the device computes g = K * reciprocal(sum) directly.
"""

import numpy as np

B, T = 512, 32768
K = 64.0
R_REFRACTORY = 4
N_CORES = 8
ROWS = B // N_CORES  # 64 rows per core
P = 128

NCHUNK = 2
RPC = ROWS // NCHUNK  # 32 rows per chunk
GS = P // RPC  # 4 partitions per row within a chunk
WC = RPC * T // P  # 8192 free width per chunk
ACT_W = 2048  # ACT tile width

_NC_CACHE: dict = {}


def _build_nc(inv_temp: float, reps: int = 1):
    from contextlib import ExitStack

    import concourse.bacc as bacc
    import concourse.tile as tile
    from concourse import mybir

    f32 = mybir.dt.float32
    nc = bacc.Bacc(
        "TRN2",
        target_bir_lowering=False,
        debug=False,
        enable_asserts=False,
        num_devices=N_CORES,
    )
    scores_h = nc.dram_tensor("scores", [ROWS, T], f32, kind="ExternalInput")
    wsum_h = nc.dram_tensor("wsum", [P, P], f32, kind="ExternalInput")
    mask_h = nc.dram_tensor("mask", [P, 1], f32, kind="ExternalInput")
    y_h = nc.dram_tensor("y", [ROWS, T], f32, kind="ExternalOutput")
    bud_h = nc.dram_tensor("budgets", [NCHUNK, P], f32, kind="ExternalOutput")

    # [nchunk, 128, Wc] flat-contiguous chunk views
    s_k = scores_h.rearrange("r (q w) -> (r q) w", w=WC).rearrange(
        "(k p) w -> k p w", p=P
    )
    y_k = y_h.rearrange("r (q w) -> (r q) w", w=WC).rearrange("(k p) w -> k p w", p=P)

    with tile.TileContext(nc) as tc, ExitStack() as ctx:
        inp = ctx.enter_context(tc.tile_pool(name="inp", bufs=2))
        sig = ctx.enter_context(tc.tile_pool(name="sig", bufs=2))
        outp = ctx.enter_context(tc.tile_pool(name="outp", bufs=2))
        stats = ctx.enter_context(tc.tile_pool(name="stats", bufs=4))
        consts = ctx.enter_context(tc.tile_pool(name="consts", bufs=1))
        psum = ctx.enter_context(tc.tile_pool(name="psum", bufs=4, space="PSUM"))

        wsum_t = consts.tile([P, P], f32)
        nc.sync.dma_start(wsum_t[:], wsum_h[:, :])
        mask_t = consts.tile([P, 1], f32)
        nc.sync.dma_start(mask_t[:], mask_h[:, :])
        # Load the sigmoid ACT table set while the first big DMA streams.
        wtile = consts.tile([P, 1], f32)
        nc.vector.memset(wtile[:], 0.0)
        nc.scalar.activation(wtile[:], wtile[:], mybir.ActivationFunctionType.Sigmoid)

        for _rep in range(reps):
            for k in range(NCHUNK):
                t_in = inp.tile([P, WC], f32, tag="in")
                nc.sync.dma_start(t_in[:], s_k[k, :, :])
                t_sig = sig.tile([P, WC], f32, tag="sig")
                ntile = WC // ACT_W
                partials = stats.tile([P, ntile], f32, tag="partials")
                for i in range(ntile):
                    sl = slice(i * ACT_W, (i + 1) * ACT_W)
                    nc.scalar.activation(
                        t_sig[:, sl],
                        t_in[:, sl],
                        mybir.ActivationFunctionType.Sigmoid,
                        scale=float(inv_temp),
                        accum_out=partials[:, i : i + 1],
                    )
                total = stats.tile([P, 1], f32, tag="total")
                nc.vector.tensor_reduce(
                    total[:],
                    partials[:],
                    axis=mybir.AxisListType.X,
                    op=mybir.AluOpType.add,
                )
                # group-sum + broadcast: bud[p] = sum of total over p's 4-group
                bud_ps = psum.tile([P, 1], f32, tag="budps")
                nc.tensor.matmul(
                    bud_ps[:], wsum_t[:], total[:, 0:1], start=True, stop=True
                )
                rb = stats.tile([P, 1], f32, tag="rb")
                nc.vector.reciprocal(rb[:], bud_ps[:])
                gm = stats.tile([P, 1], f32, tag="gm")  # K/b with row-start zeroing
                nc.vector.tensor_scalar(
                    gm[:],
                    rb[:],
                    mask_t[:, 0:1],
                    K,
                    op0=mybir.AluOpType.mult,
                    op1=mybir.AluOpType.mult,
                )
                t_out = outp.tile([P, WC], f32, tag="out")
                # plain single-op TS with precomputed g keeps 2x mode
                g = stats.tile([P, 1], f32, tag="g")
                nc.vector.tensor_scalar_mul(g[:], rb[:], K)
                nc.vector.tensor_scalar_mul(t_out[:, :], t_sig[:, :], g[:, 0:1])
                nc.vector.tensor_mul(t_out[:, 0:1], t_sig[:, 0:1], gm[:, 0:1])
                nc.sync.dma_start(y_k[k, :, :], t_out[:])
                # export raw row sums (off critical path)
                bud = stats.tile([P, 1], f32, tag="bud")
                nc.vector.tensor_copy(bud[:], bud_ps[:])
                nc.gpsimd.dma_start(bud_h[k : k + 1, :], bud[:, 0:1])
    nc.compile()
    return nc


def _get_nc(inv_temp: float, reps: int = 1):
    key = (round(float(inv_temp), 9), reps)
    if key not in _NC_CACHE:
        _NC_CACHE[key] = _build_nc(inv_temp, reps)
    return _NC_CACHE[key]


def _wsum_matrix() -> np.ndarray:
    # wsum[k, m] = 1 iff k//GS == m//GS: sums each row's GS partitions and
    # broadcasts back to all of them — one matmul does the whole reduction.
    return np.kron(np.eye(P // GS, dtype=np.float32), np.ones((GS, GS), np.float32))


def _mask_matrix() -> np.ndarray:
    # 0 at partitions holding a row start (p % GS == 0), else 1
    m = np.ones((P, 1), np.float32)
    m[0::GS, 0] = 0.0
    return m


def _temp_from_log(log_temperature) -> np.float32:
    lt = np.float32(np.asarray(log_temperature, dtype=np.float32).reshape(()))
    return np.float32(np.clip(np.exp(lt, dtype=np.float32), 0.1, 10.0))


def _reference_fallback(scores: np.ndarray, temp: np.float32) -> np.ndarray:
    # Exact general-case evaluation (mirrors reference.py in fp32 numpy).
    y = 1.0 / (1.0 + np.exp(-(scores / temp), dtype=np.float32))
    y = y.astype(np.float32)
    budget = np.clip(np.sum(y, axis=1, keepdims=True, dtype=np.float32), 1e-6, None)
    y = y * np.minimum(np.float32(K) / budget, np.float32(1.0))
    t = scores.shape[1]
    for d in range(1, min(R_REFRACTORY + 1, t)):
        shift = np.roll(y, -d, axis=1)
        y = y * np.minimum(2.0 / (1.0 + y + shift), 1.0).astype(np.float32)
    y = y.astype(np.float32)
    y[:, 0] = 0.0
    return y


def kernel(scores: np.ndarray, log_temperature: np.ndarray) -> np.ndarray:
    from concourse.bass_utils import run_bass_kernel_spmd

    scores = np.ascontiguousarray(scores, dtype=np.float32)
    assert scores.shape == (B, T), scores.shape
    temp = _temp_from_log(log_temperature)
    inv_temp = np.float32(1.0) / temp

    nc = _get_nc(float(inv_temp))
    wsum = _wsum_matrix()
    mask = _mask_matrix()
    in_maps = [
        {"scores": scores[c * ROWS : (c + 1) * ROWS], "wsum": wsum, "mask": mask}
        for c in range(N_CORES)
    ]
    res = run_bass_kernel_spmd(nc, in_maps, list(range(N_CORES))).results
    y = np.concatenate([res[c]["y"] for c in range(N_CORES)], axis=0)
    # budgets[k, GS*j] = raw sum of row RPC*k + j (per core)
    budgets = np.concatenate(
        [res[c]["budgets"][:, 0::GS].reshape(-1) for c in range(N_CORES)]
    )

    # Damping is an exact fp32 identity iff every row budget >= 2K (see module
    # docstring); 256 adds 2x margin over the required 128. If violated (never,
    # for randn-scale inputs), recompute everything faithfully on the host.
    if not np.all(budgets >= 256.0):
        return _reference_fallback(scores, temp)
    return y



# revision 2
# speedup vs baseline: 1.7810x; 1.7810x over previous
"""Trainium2 Bass kernel: nn_DifferentiableSelector (soft top-K w/ refractory damping).

Data-parallel over batch: 512 rows -> 64 rows/core on 8 NeuronCores.

The kernel is HBM-bandwidth bound (~358 GB/s/core spec), so I/O is 16-bit:
scores stream in as fp16 (host downcast; |s| <= ~6 so worst-case input
rounding is 2^-11 relative -> <=0.3% sigmoid error) and y streams out as
bf16 (<=0.2% representation error, no subnormal risk at y ~ 1e-5). That
halves HBM traffic vs f32 (8.4 MB/core/rep instead of 16.8 MB) and roughly
halves wall time; total worst-case elementwise error ~0.7% vs the 2e-2 gate.

Device layout ("two contiguous row-chunks"): each core's [64, 32768] block is
split into 2 contiguous address-range chunks of 32 rows. Chunk k, viewed as
[128, 8192], holds rows 32k..32k+31 with row 32k+j on partitions
[4j, 4j+4) — so every DMA is one fully-contiguous 2MB transfer (measured
6-30x faster on this target than partition-interleaved patterns), and chunk
k+1's input DMA overlaps chunk k's compute while chunk k's output DMA overlaps
chunk k+1's compute. Per chunk: one full-width 8192 ACT sigmoid (fp16 in,
bf16 out) with fused row-partial accumulation (accum_out) directly yields the
per-partition sums; one PE matmul against a 0/1 block matrix group-sums +
broadcasts the row budgets; reciprocal straight from PSUM; then one
full-width (even-length, 4x-mode eligible) DVE tensor_scalar scale pass
bf16->bf16; column 0 of each row is then overwritten via a masked
per-partition factor to implement y[:, 0] = 0.

Math: y0 = sigmoid(scores/temp); budget_r = clip(sum_i y0[r,i], 1e-6);
y = y0 * min(K/budget, 1); then R=4 damping iters
y *= min(2/(1+y+roll(y,-d)), 1); y[:,0] = 0.

Damping-identity property (load-bearing): if budget_r >= 2K = 128 for every
row, then min(K/budget,1) <= 0.5 (correctly-rounded fp32 div), so every
y <= 0.5, so s = fl(y[i]+y[i+d]) <= 1, fl(1+s) <= 2, fl(2/(1+s)) >= 1, and
min(2/(1+s), 1.0) == 1.0 *exactly*; y*1.0 is bitwise identity. Inductively the
whole damping loop is an exact fp32 no-op. For N(0,1)-like scores,
budget ~ T/2 = 16384 (margin ~128x over the threshold). The device exports the
raw per-row sums; the host checks sum >= 256 for every row and otherwise falls
back to a full numpy evaluation of the reference semantics (exact for
arbitrary inputs; never taken for the spec'd input distribution). The same
check makes clip(budget, 1e-6) and min(K/budget, 1) identities on the device
path, so the device computes g = K * reciprocal(sum) directly.
"""

import numpy as np

B, T = 512, 32768
K = 64.0
R_REFRACTORY = 4
N_CORES = 8
ROWS = B // N_CORES  # 64 rows per core
P = 128

NCHUNK = 2
RPC = ROWS // NCHUNK  # 32 rows per chunk
GS = P // RPC  # 4 partitions per row within a chunk
WC = RPC * T // P  # 8192 free width per chunk

_NC_CACHE: dict = {}


def _build_nc(inv_temp: float, reps: int = 1):
    from contextlib import ExitStack

    import concourse.bacc as bacc
    import concourse.tile as tile
    from concourse import mybir

    f32 = mybir.dt.float32
    f16 = mybir.dt.float16
    bf16 = mybir.dt.bfloat16
    nc = bacc.Bacc(
        "TRN2",
        target_bir_lowering=False,
        debug=False,
        enable_asserts=False,
        num_devices=N_CORES,
    )
    scores_h = nc.dram_tensor("scores", [ROWS, T], f16, kind="ExternalInput")
    wsum_h = nc.dram_tensor("wsum", [P, P], f32, kind="ExternalInput")
    mask_h = nc.dram_tensor("mask", [P, 1], f32, kind="ExternalInput")
    y_h = nc.dram_tensor("y", [ROWS, T], bf16, kind="ExternalOutput")
    bud_h = nc.dram_tensor("budgets", [NCHUNK, P], f32, kind="ExternalOutput")

    # [nchunk, 128, Wc] flat-contiguous chunk views
    s_k = scores_h.rearrange("r (q w) -> (r q) w", w=WC).rearrange(
        "(k p) w -> k p w", p=P
    )
    y_k = y_h.rearrange("r (q w) -> (r q) w", w=WC).rearrange("(k p) w -> k p w", p=P)

    with tile.TileContext(nc) as tc, ExitStack() as ctx:
        inp = ctx.enter_context(tc.tile_pool(name="inp", bufs=2))
        sig = ctx.enter_context(tc.tile_pool(name="sig", bufs=2))
        outp = ctx.enter_context(tc.tile_pool(name="outp", bufs=2))
        stats = ctx.enter_context(tc.tile_pool(name="stats", bufs=4))
        consts = ctx.enter_context(tc.tile_pool(name="consts", bufs=1))
        psum = ctx.enter_context(tc.tile_pool(name="psum", bufs=4, space="PSUM"))

        wsum_t = consts.tile([P, P], f32)
        nc.sync.dma_start(wsum_t[:], wsum_h[:, :])
        mask_t = consts.tile([P, 1], f32)
        nc.sync.dma_start(mask_t[:], mask_h[:, :])
        # Load the sigmoid ACT table set while the first big DMA streams.
        wtile = consts.tile([P, 1], f32)
        nc.vector.memset(wtile[:], 0.0)
        nc.scalar.activation(wtile[:], wtile[:], mybir.ActivationFunctionType.Sigmoid)

        for _rep in range(reps):
            for k in range(NCHUNK):
                t_in = inp.tile([P, WC], f16, tag="in")
                nc.sync.dma_start(t_in[:], s_k[k, :, :])
                t_sig = sig.tile([P, WC], bf16, tag="sig")
                total = stats.tile([P, 1], f32, tag="total")
                nc.scalar.activation(
                    t_sig[:, :],
                    t_in[:, :],
                    mybir.ActivationFunctionType.Sigmoid,
                    scale=float(inv_temp),
                    accum_out=total[:, 0:1],
                )
                # group-sum + broadcast: bud[p] = sum of total over p's 4-group
                bud_ps = psum.tile([P, 1], f32, tag="budps")
                nc.tensor.matmul(
                    bud_ps[:], wsum_t[:], total[:, 0:1], start=True, stop=True
                )
                rb = stats.tile([P, 1], f32, tag="rb")
                nc.vector.reciprocal(rb[:], bud_ps[:])
                gm = stats.tile([P, 1], f32, tag="gm")  # K/b with row-start zeroing
                nc.vector.tensor_scalar(
                    gm[:],
                    rb[:],
                    mask_t[:, 0:1],
                    K,
                    op0=mybir.AluOpType.mult,
                    op1=mybir.AluOpType.mult,
                )
                t_out = outp.tile([P, WC], bf16, tag="out")
                # plain single-op TS with precomputed g keeps 4x mode
                g = stats.tile([P, 1], f32, tag="g")
                nc.vector.tensor_scalar_mul(g[:], rb[:], K)
                nc.vector.tensor_scalar_mul(t_out[:, :], t_sig[:, :], g[:, 0:1])
                nc.vector.tensor_scalar_mul(t_out[:, 0:1], t_sig[:, 0:1], gm[:, 0:1])
                nc.sync.dma_start(y_k[k, :, :], t_out[:])
                # export raw row sums (off critical path)
                bud = stats.tile([P, 1], f32, tag="bud")
                nc.vector.tensor_copy(bud[:], bud_ps[:])
                nc.gpsimd.dma_start(bud_h[k : k + 1, :], bud[:, 0:1])
    nc.compile()
    return nc


def _get_nc(inv_temp: float, reps: int = 1):
    key = (round(float(inv_temp), 9), reps)
    if key not in _NC_CACHE:
        _NC_CACHE[key] = _build_nc(inv_temp, reps)
    return _NC_CACHE[key]


def _wsum_matrix() -> np.ndarray:
    # wsum[k, m] = 1 iff k//GS == m//GS: sums each row's GS partitions and
    # broadcasts back to all of them — one matmul does the whole reduction.
    return np.kron(np.eye(P // GS, dtype=np.float32), np.ones((GS, GS), np.float32))


def _mask_matrix() -> np.ndarray:
    # 0 at partitions holding a row start (p % GS == 0), else 1
    m = np.ones((P, 1), np.float32)
    m[0::GS, 0] = 0.0
    return m


def _temp_from_log(log_temperature) -> np.float32:
    lt = np.float32(np.asarray(log_temperature, dtype=np.float32).reshape(()))
    return np.float32(np.clip(np.exp(lt, dtype=np.float32), 0.1, 10.0))


def _in_maps(scores: np.ndarray) -> list:
    s16 = np.ascontiguousarray(scores, dtype=np.float32).astype(np.float16)
    wsum = _wsum_matrix()
    mask = _mask_matrix()
    return [
        {"scores": s16[c * ROWS : (c + 1) * ROWS], "wsum": wsum, "mask": mask}
        for c in range(N_CORES)
    ]


def _reference_fallback(scores: np.ndarray, temp: np.float32) -> np.ndarray:
    # Exact general-case evaluation (mirrors reference.py in fp32 numpy).
    y = 1.0 / (1.0 + np.exp(-(scores / temp), dtype=np.float32))
    y = y.astype(np.float32)
    budget = np.clip(np.sum(y, axis=1, keepdims=True, dtype=np.float32), 1e-6, None)
    y = y * np.minimum(np.float32(K) / budget, np.float32(1.0))
    t = scores.shape[1]
    for d in range(1, min(R_REFRACTORY + 1, t)):
        shift = np.roll(y, -d, axis=1)
        y = y * np.minimum(2.0 / (1.0 + y + shift), 1.0).astype(np.float32)
    y = y.astype(np.float32)
    y[:, 0] = 0.0
    return y


def kernel(scores: np.ndarray, log_temperature: np.ndarray) -> np.ndarray:
    from concourse.bass_utils import run_bass_kernel_spmd

    scores = np.ascontiguousarray(scores, dtype=np.float32)
    assert scores.shape == (B, T), scores.shape
    temp = _temp_from_log(log_temperature)
    inv_temp = np.float32(1.0) / temp

    nc = _get_nc(float(inv_temp))
    in_maps = _in_maps(scores)
    res = run_bass_kernel_spmd(nc, in_maps, list(range(N_CORES))).results
    y = np.concatenate(
        [np.asarray(res[c]["y"]).astype(np.float32) for c in range(N_CORES)], axis=0
    )
    # budgets[k, GS*j] = raw sum of row RPC*k + j (per core)
    budgets = np.concatenate(
        [np.asarray(res[c]["budgets"])[:, 0::GS].reshape(-1) for c in range(N_CORES)]
    )

    # Damping is an exact fp32 identity iff every row budget >= 2K (see module
    # docstring); 256 adds 2x margin over the required 128. If violated (never,
    # for randn-scale inputs), recompute everything faithfully on the host.
    if not np.all(budgets >= 256.0):
        return _reference_fallback(scores, temp)
    return y


# revision 3
# speedup vs baseline: 2.9704x; 1.6678x over previous
"""Trainium2 Bass kernel: nn_DifferentiableSelector (soft top-K w/ refractory damping).

Data-parallel over batch: 512 rows -> 64 rows/core on 8 NeuronCores.

The kernel is HBM-bandwidth bound, so I/O is 16-bit: scores stream in as
fp16 (host downcast; |s| <= ~6 so worst-case input rounding is 2^-11
relative -> <=0.3% sigmoid error) and y streams out as bf16 (<=0.2%
representation error, no subnormal risk at y ~ 1e-5). That halves HBM
traffic vs f32 (8.4 MB/core/rep instead of 16.8 MB). Measured ablations on
this target: the pure DMA loop runs ~14-16.5us/rep, and a gpsimd (SWDGE)
512B budget-export DMA per chunk was worth ~18us/rep of backpressure — so
the device path exports nothing but y, and all correctness guards moved to
the host (see below).

Device layout ("two contiguous row-chunks"): each core's [64, 32768] block is
split into 2 contiguous address-range chunks of 32 rows. Chunk k, viewed as
[128, 8192], holds rows 32k..32k+31 with row 32k+j on partitions
[4j, 4j+4) — so every DMA is one fully-contiguous 2MB transfer (measured
6-30x faster on this target than partition-interleaved patterns). Input DMAs
issue from the ACT HWDGE ring and output DMAs from the SP HWDGE ring so a
y-store waiting on compute never heads-of-line-blocks the next score-load
(measured ~2-3us/rep). Per chunk: one full-width 8192 ACT sigmoid (fp16 in,
bf16 out) with fused row-partial accumulation (accum_out) directly yields
the per-partition sums; one PE matmul against a 0/1 block matrix group-sums
+ broadcasts the row budgets to PSUM; DVE reciprocal; then one full-width
(even-length, 4x-mode eligible) two-op DVE tensor_scalar pass computes
y = (sig * rb) * K in bf16. The y[:,0] = 0 column fix happens on the host
after the f32 upcast (512 stores, free) instead of costing a DVE op.

Math: y0 = sigmoid(scores/temp); budget_r = clip(sum_i y0[r,i], 1e-6);
y = y0 * min(K/budget, 1); then R=4 damping iters
y *= min(2/(1+y+roll(y,-d)), 1); y[:,0] = 0.

Damping-identity property (load-bearing): if budget_r >= 2K = 128 for every
row, then min(K/budget,1) <= 0.5 (correctly-rounded fp32 div), so every
y <= 0.5, so s = fl(y[i]+y[i+d]) <= 1, fl(1+s) <= 2, fl(2/(1+s)) >= 1, and
min(2/(1+s), 1.0) == 1.0 *exactly*; y*1.0 is bitwise identity. Inductively
the whole damping loop is an exact fp32 no-op, and clip(budget, 1e-6) and
min(K/budget, 1) are identities too, so the device computes
g = K * reciprocal(sum) directly.

Host-side guard (no device cost): sigmoid(s/temp) >= 0.5 iff s >= 0 (temp>0),
so budget_r >= 0.5 * #{s >= 0 in row r} exactly in reals, and the fp32 sum's
relative error over 32768 terms is <~2e-3. The host counts nonnegative
scores per row and requires >= 2048 (=> true budget >= ~1023, an 8x margin
over the required 128, and ~158 sigma below the randn mean of 16384 — never
taken for the spec'd distribution). On failure it falls back to a full numpy
evaluation of the reference semantics (exact for arbitrary inputs).
"""

import numpy as np

B, T = 512, 32768
K = 64.0
R_REFRACTORY = 4
N_CORES = 8
ROWS = B // N_CORES  # 64 rows per core
P = 128

NCHUNK = 2
RPC = ROWS // NCHUNK  # 32 rows per chunk
GS = P // RPC  # 4 partitions per row within a chunk
WC = RPC * T // P  # 8192 free width per chunk

_NC_CACHE: dict = {}


def _build_nc(inv_temp: float, reps: int = 1, nchunk: int = NCHUNK):
    from contextlib import ExitStack

    import concourse.bacc as bacc
    import concourse.tile as tile
    from concourse import mybir

    f32 = mybir.dt.float32
    f16 = mybir.dt.float16
    bf16 = mybir.dt.bfloat16
    rpc = ROWS // nchunk
    gs = P // rpc
    wc = rpc * T // P

    nc = bacc.Bacc(
        "TRN2",
        target_bir_lowering=False,
        debug=False,
        enable_asserts=False,
        num_devices=N_CORES,
    )
    scores_h = nc.dram_tensor("scores", [ROWS, T], f16, kind="ExternalInput")
    wsum_h = nc.dram_tensor("wsum", [P, P], f32, kind="ExternalInput")
    y_h = nc.dram_tensor("y", [ROWS, T], bf16, kind="ExternalOutput")

    # [nchunk, 128, wc] flat-contiguous chunk views
    s_k = scores_h.rearrange("r (q w) -> (r q) w", w=wc).rearrange(
        "(k p) w -> k p w", p=P
    )
    y_k = y_h.rearrange("r (q w) -> (r q) w", w=wc).rearrange("(k p) w -> k p w", p=P)

    with tile.TileContext(nc) as tc, ExitStack() as ctx:
        inp = ctx.enter_context(tc.tile_pool(name="inp", bufs=2))
        sig = ctx.enter_context(tc.tile_pool(name="sig", bufs=2))
        outp = ctx.enter_context(tc.tile_pool(name="outp", bufs=2))
        stats = ctx.enter_context(tc.tile_pool(name="stats", bufs=4))
        consts = ctx.enter_context(tc.tile_pool(name="consts", bufs=1))
        psum = ctx.enter_context(tc.tile_pool(name="psum", bufs=4, space="PSUM"))

        wsum_t = consts.tile([P, P], f32)
        nc.sync.dma_start(wsum_t[:], wsum_h[:, :])
        # Load the sigmoid ACT table set while the first big DMA streams.
        wtile = consts.tile([P, 1], f32)
        nc.vector.memset(wtile[:], 0.0)
        nc.scalar.activation(wtile[:], wtile[:], mybir.ActivationFunctionType.Sigmoid)

        for _rep in range(reps):
            for k in range(nchunk):
                t_in = inp.tile([P, wc], f16, tag="in")
                nc.scalar.dma_start(t_in[:], s_k[k, :, :])
                t_sig = sig.tile([P, wc], bf16, tag="sig")
                total = stats.tile([P, 1], f32, tag="total")
                nc.scalar.activation(
                    t_sig[:, :],
                    t_in[:, :],
                    mybir.ActivationFunctionType.Sigmoid,
                    scale=float(inv_temp),
                    accum_out=total[:, 0:1],
                )
                # group-sum + broadcast: bud[p] = sum of total over p's 4-group
                bud_ps = psum.tile([P, 1], f32, tag="budps")
                nc.tensor.matmul(
                    bud_ps[:], wsum_t[:], total[:, 0:1], start=True, stop=True
                )
                rb = stats.tile([P, 1], f32, tag="rb")
                nc.vector.reciprocal(rb[:], bud_ps[:])
                t_out = outp.tile([P, wc], bf16, tag="out")
                nc.vector.tensor_scalar(
                    t_out[:, :],
                    t_sig[:, :],
                    rb[:, 0:1],
                    K,
                    op0=mybir.AluOpType.mult,
                    op1=mybir.AluOpType.mult,
                )
                nc.sync.dma_start(y_k[k, :, :], t_out[:])
    nc.compile()
    return nc


def _get_nc(inv_temp: float, reps: int = 1):
    key = (round(float(inv_temp), 9), reps)
    if key not in _NC_CACHE:
        _NC_CACHE[key] = _build_nc(inv_temp, reps)
    return _NC_CACHE[key]


def _wsum_matrix() -> np.ndarray:
    # wsum[k, m] = 1 iff k//GS == m//GS: sums each row's GS partitions and
    # broadcasts back to all of them — one matmul does the whole reduction.
    return np.kron(np.eye(P // GS, dtype=np.float32), np.ones((GS, GS), np.float32))


def _temp_from_log(log_temperature) -> np.float32:
    lt = np.float32(np.asarray(log_temperature, dtype=np.float32).reshape(()))
    return np.float32(np.clip(np.exp(lt, dtype=np.float32), 0.1, 10.0))


def _in_maps(scores: np.ndarray) -> list:
    s16 = np.ascontiguousarray(scores, dtype=np.float32).astype(np.float16)
    wsum = _wsum_matrix()
    return [
        {"scores": s16[c * ROWS : (c + 1) * ROWS], "wsum": wsum}
        for c in range(N_CORES)
    ]


def _reference_fallback(scores: np.ndarray, temp: np.float32) -> np.ndarray:
    # Exact general-case evaluation (mirrors reference.py in fp32 numpy).
    y = 1.0 / (1.0 + np.exp(-(scores / temp), dtype=np.float32))
    y = y.astype(np.float32)
    budget = np.clip(np.sum(y, axis=1, keepdims=True, dtype=np.float32), 1e-6, None)
    y = y * np.minimum(np.float32(K) / budget, np.float32(1.0))
    t = scores.shape[1]
    for d in range(1, min(R_REFRACTORY + 1, t)):
        shift = np.roll(y, -d, axis=1)
        y = y * np.minimum(2.0 / (1.0 + y + shift), 1.0).astype(np.float32)
    y = y.astype(np.float32)
    y[:, 0] = 0.0
    return y


def kernel(scores: np.ndarray, log_temperature: np.ndarray) -> np.ndarray:
    from concourse.bass_utils import run_bass_kernel_spmd

    scores = np.ascontiguousarray(scores, dtype=np.float32)
    assert scores.shape == (B, T), scores.shape
    temp = _temp_from_log(log_temperature)
    inv_temp = np.float32(1.0) / temp

    # Host guard for the damping-identity fast path (see module docstring):
    # every row needs >= 2048 nonnegative scores => true budget >= ~1023.
    nonneg = np.count_nonzero(scores >= 0, axis=1)
    if not np.all(nonneg >= 2048):
        return _reference_fallback(scores, temp)

    nc = _get_nc(float(inv_temp))
    res = run_bass_kernel_spmd(nc, _in_maps(scores), list(range(N_CORES))).results
    y = np.concatenate(
        [np.asarray(res[c]["y"]).astype(np.float32) for c in range(N_CORES)], axis=0
    )
    y[:, 0] = 0.0
    return y
